# revision 23
# baseline (speedup 1.0000x reference)
"""Trainium2 Bass kernel for LowRankTriLinearFusionAttn.

Math (per sample b):
  g  = relu(LN(h_g  @ Wg.T + bg))          (256)
  d2 = relu(LN(h_2d @ W2.T + b2))          (256)
  d3 = relu(LN(h_3d @ W3.T + b3))          (256)
  z_r[b,r,:] = (g U_r^T) * (d2 V_r^T) * (d3 S_r^T)     r in 0..15
  beta = softmax(relu([h_g|h_2d|h_3d] @ Wa1.T + ba1) @ Wa2.T + ba2)
  z[b,:] = sum_r beta[b,r] * z_r[b,r,:]

Sharding: pure data parallel over 8 NeuronCores (batch 8192 -> 1024/core).
Weights are replicated; the host pre-packs them transposed (contraction dim
on partitions) and cast to bf16 — standard offline model packing.
Activations are cast to bf16 by SWDGE DMA and transposed on the PE.

Emission order is chosen so the PE instruction stream is dense:
x-transposes -> projections -> attention l1 -> gT transposes -> a2/softmax
-> rank expansion. LayerNorm / softmax / trilinear elementwise chains run
on DVE/ACT/GpSimd concurrently with later PE groups.
"""

import sys
import types

import numpy as np
import ml_dtypes

import concourse.bass as bass
import concourse.tile as tile
from concourse import bacc
from concourse import mybir
from concourse.bass import ts
from concourse.bass_utils import run_bass_kernel_spmd
from concourse.masks import make_identity
import bass_rust


def _ensure_ntff_hook():
    """Provide antenv.axon_hooks if the image's antenv stub lacks it, so
    run_bass_kernel_spmd(trace=True) can capture NTFF profiles under axon."""
    try:
        import antenv.axon_hooks  # noqa: F401
        return
    except ImportError:
        pass
    try:
        from trn_agent_boot.trn_boot import _ntff_profile_via_ctypes

        hook = _ntff_profile_via_ctypes("/opt/axon/libaxon_pjrt.so")
    except Exception:
        hook = None
    mod = types.ModuleType("antenv.axon_hooks")
    _state = {"hook": hook}
    mod.get_axon_ntff_profile_hook = lambda: _state["hook"]
    mod.set_axon_ntff_profile_hook = lambda h: _state.update(hook=h)
    sys.modules["antenv.axon_hooks"] = mod


_ensure_ntff_hook()

BF16 = mybir.dt.bfloat16
F32 = mybir.dt.float32
AF = mybir.ActivationFunctionType
OP = mybir.AluOpType

N_CORES = 8
B = 8192
D_G, D_2D, D_3D = 512, 768, 1024
D_CAT = D_G + D_2D + D_3D  # 2304
D_F, RANK, ATTN_H = 256, 16, 512
RD = RANK * D_F  # 4096
P = 128

BC = B // N_CORES           # 1024 samples per core
NBT = BC // P               # 8 batch tiles per core
KD = [D_G // P, D_2D // P, D_3D // P]   # k-tiles per modality: 4, 6, 8
KOFF = [0, KD[0], KD[0] + KD[1]]        # xt chunk offsets: 0, 4, 10
NK = D_CAT // P             # 18
NH = ATTN_H // P            # 4
NCH = RD // 512             # 8 chunks of 512 in the rank-expanded dim
KF = D_F // P               # 2 k-tiles for the 256-dim contraction
EPS = 1e-5


def build_kernel(bc=BC):
    assert bc % 512 == 0
    nbt = bc // P
    nc = bacc.Bacc("TRN2", debug=False)

    # ---- external I/O (per-core shapes) ----
    h_g = nc.dram_tensor("h_g", [bc, D_G], F32, kind="ExternalInput").ap()
    h_2d = nc.dram_tensor("h_2d", [bc, D_2D], F32, kind="ExternalInput").ap()
    h_3d = nc.dram_tensor("h_3d", [bc, D_3D], F32, kind="ExternalInput").ap()
    wc_t = nc.dram_tensor("wc_t", [D_CAT, D_F], BF16, kind="ExternalInput").ap()
    uvs_t = nc.dram_tensor("uvs_t", [6 * P, RD], BF16, kind="ExternalInput").ap()
    wa1_t = nc.dram_tensor("wa1_t", [D_CAT, ATTN_H], BF16, kind="ExternalInput").ap()
    wa2_t = nc.dram_tensor("wa2_t", [ATTN_H, RANK], BF16, kind="ExternalInput").ap()
    consts_f = nc.dram_tensor("consts_f", [P, 16], F32, kind="ExternalInput").ap()
    consts_b = nc.dram_tensor("consts_b", [4, D_F], BF16, kind="ExternalInput").ap()
    z_out = nc.dram_tensor("z", [bc, D_F], F32, kind="ExternalOutput").ap()

    from contextlib import ExitStack

    with tile.TileContext(nc) as tc, ExitStack() as ctx:
        consts = ctx.enter_context(tc.tile_pool(name="consts", bufs=1))
        wpool = ctx.enter_context(tc.tile_pool(name="w", bufs=1))
        xtp = ctx.enter_context(tc.tile_pool(name="xt", bufs=1))
        gtp = ctx.enter_context(tc.tile_pool(name="gt", bufs=1))
        sp = ctx.enter_context(tc.tile_pool(name="sp", bufs=16))
        zp = ctx.enter_context(tc.tile_pool(name="zacc", bufs=3))
        pp = ctx.enter_context(tc.tile_pool(name="ps", bufs=8, space="PSUM"))
        xnp = tc.tile_pool(name="xn", bufs=1)     # scoped: closed after stage 1
        xnpool = xnp.__enter__()

        # ---------- constants ----------
        identity = consts.tile([P, P], BF16, tag="ident")
        make_identity(nc, identity)
        ones_row = consts.tile([1, P], BF16, tag="ones")
        nc.vector.memset(ones_row, 1.0)
        eps_t = consts.tile([P, 1], F32, tag="eps")
        nc.vector.memset(eps_t, EPS)
        cf_sb = consts.tile([P, 16], F32, tag="cf")  # ba1(4) lnw(6) lnb(6)
        nc.sync.dma_start(out=cf_sb, in_=consts_f)
        cb_sb = consts.tile([1, 4, D_F], BF16, tag="cb")  # bg b2 b3 ba2pad
        nc.sync.dma_start(out=cb_sb, in_=consts_b.rearrange("(o m) n -> o m n", o=1))
        ba1_sb = cf_sb[:, 0:NH]
        lnw_sb = cf_sb[:, 4:10]
        lnb_sb = cf_sb[:, 10:16]
        bias_sb = cb_sb[:, 0:3, :]
        ba2_sb = cb_sb[:, 3, 0:RANK]

        # ---------- weights ----------
        wc_sb = wpool.tile([P, NK, D_F], BF16, tag="wc")
        nc.sync.dma_start(out=wc_sb, in_=wc_t.rearrange("(t p) n -> p t n", p=P))
        wa1_sb = wpool.tile([P, NK, ATTN_H], BF16, tag="wa1")
        d_wa1 = nc.sync.dma_start(
            out=wa1_sb, in_=wa1_t.rearrange("(t p) n -> p t n", p=P)
        )
        wa2_sb = wpool.tile([P, NH, RANK], BF16, tag="wa2")
        d_wa2 = nc.sync.dma_start(
            out=wa2_sb, in_=wa2_t.rearrange("(t p) n -> p t n", p=P)
        )
        uvs_sb = wpool.tile([P, 6, RD], BF16, tag="uvs")
        d_uvs = nc.sync.dma_start(
            out=uvs_sb, in_=uvs_t.rearrange("(t p) n -> p t n", p=P)
        )

        # ---------- input cast (per b-tile/modality SWDGE cast-DMAs; unique
        # dest tiles keep each pseudo-DMA at zero sync waits, and small DMAs
        # give the first x-transposes low latency) ----------
        xin = [h_g, h_2d, h_3d]
        xn = [[None] * 3 for _ in range(nbt)]
        xn_dmas = []
        for t in range(nbt):
            for m in range(3):
                t_ = xnpool.tile(
                    [P, KD[m] * P], BF16, tag=f"xn{t}_{m}", name=f"xn{t}_{m}"
                )
                d = nc.gpsimd.dma_start(out=t_, in_=xin[m][ts(t, P), :])
                xn_dmas.append(d)
                xn[t][m] = t_
        # let the activation loads win the HBM race; big weights follow
        bass_rust.add_dep_helper(
            d_wa1.ins, xn_dmas[-1].ins, reason="wa1 after xn loads"
        )
        bass_rust.add_dep_helper(
            d_wa2.ins, xn_dmas[-1].ins, reason="wa2 after xn loads"
        )
        bass_rust.add_dep_helper(d_uvs.ins, d_wa1.ins, reason="uvs after wa1")

        # ---------- x transposes: xt[k] = [128, bc] bf16 (x^T chunks) -------
        # 4 PE transposes write one [128, 512] PSUM tile -> single DVE evict
        xt = [
            xtp.tile([P, bc], BF16, tag=f"xt{k}", name=f"xt{k}") for k in range(NK)
        ]
        for half in range(nbt // 4):
            for m in range(3):
                for k in range(KD[m]):
                    tp = pp.tile([P, 512], BF16, tag="ps", name="tp")
                    for tt in range(4):
                        t = half * 4 + tt
                        nc.tensor.transpose(
                            tp[:, ts(tt, P)], xn[t][m][:, ts(k, P)], identity
                        )
                    nc.vector.tensor_copy(
                        xt[KOFF[m] + k][:, ts(half, 512)], tp
                    )
        xnp.__exit__(None, None, None)

        # ---------- projections + LN chains ----------
        ups = []  # u tiles: normalized (pre-affine/relu) proj, [128, 256] bf16
        for t in range(nbt):
            for m in range(3):
                ps = pp.tile([P, D_F], F32, tag="ps", name="ps_proj")
                for k in range(KD[m]):
                    nc.tensor.matmul(
                        ps,
                        lhsT=xt[KOFF[m] + k][:, ts(t, P)],
                        rhs=wc_sb[:, KOFF[m] + k, :],
                        start=(k == 0),
                        stop=False,
                    )
                nc.tensor.matmul(
                    ps, lhsT=ones_row, rhs=bias_sb[:, m, :], start=False, stop=True
                )
                stats = sp.tile([P, 6], F32, tag="stats", name="stats")
                nc.vector.bn_stats(stats, ps)
                mv = sp.tile([P, 2], F32, tag="mv", name="mv")
                nc.vector.bn_aggr(mv, stats)
                sd = sp.tile([P, 1], F32, tag="sd", name="sd")
                nc.scalar.activation(sd, mv[:, 1:2], AF.Sqrt, bias=eps_t, scale=1.0)
                rstd = sp.tile([P, 1], F32, tag="rstd", name="rstd")
                nc.vector.reciprocal(rstd, sd)
                u = gtp.tile([P, D_F], BF16, tag=f"u{t}_{m}", name=f"u{t}_{m}")
                nc.vector.tensor_scalar(
                    out=u,
                    in0=ps,
                    scalar1=mv[:, 0:1],
                    scalar2=rstd,
                    op0=OP.subtract,
                    op1=OP.mult,
                )
                ups.append(u)

        # ---------- attention layer 1 / gT transposes / softmax / rank ----
        # Emitted in "waves": each 512-sample attention slice is followed by
        # the gT transposes, a2+softmax, and rank stage of its 4 b-tiles, so
        # the DVE/ACT trilinear work overlaps the next wave's PE matmuls.
        a1t = wpool.tile([P, NH, bc], BF16, tag="a1t")  # relu(a1)^T
        gt = [[[None] * KF for _ in range(3)] for _ in range(nbt)]
        betas = [None] * nbt
        cpp = tc.tile_pool(name="cp", bufs=6)
        cp = cpp.__enter__()

        def emit_attn_l1(c):
            for h in range(NH):
                ps = pp.tile([P, 512], F32, tag="ps", name="ps_a1")
                for k in range(NK):
                    nc.tensor.matmul(
                        ps,
                        lhsT=wa1_sb[:, k, ts(h, P)],
                        rhs=xt[k][:, ts(c, 512)],
                        start=(k == 0),
                        stop=(k == NK - 1),
                    )
                nc.scalar.activation(
                    a1t[:, h, ts(c, 512)],
                    ps,
                    AF.Relu,
                    bias=ba1_sb[:, h : h + 1],
                    scale=1.0,
                )

        def emit_gtt(t):
            for m in range(3):
                u = ups[t * 3 + m]
                for j in range(KF):
                    tp = pp.tile([P, P], BF16, tag="ps", name="tpg")
                    nc.tensor.transpose(tp, u[:, ts(j, P)], identity)
                    g = gtp.tile(
                        [P, P], BF16, tag=f"gt{t}_{m}{j}", name=f"g{t}_{m}{j}"
                    )
                    col = m * KF + j
                    # ln affine (feature = partition here) then relu, on DVE
                    nc.vector.tensor_scalar(
                        out=g,
                        in0=tp,
                        scalar1=lnw_sb[:, col : col + 1],
                        scalar2=lnb_sb[:, col : col + 1],
                        op0=OP.mult,
                        op1=OP.add,
                    )
                    nc.vector.tensor_scalar_max(out=g, in0=g, scalar1=0.0)
                    gt[t][m][j] = g

        def emit_a2_softmax(t):
            ps = pp.tile([P, RANK], F32, tag="ps", name="ps_a2")
            for k in range(NH):
                nc.tensor.matmul(
                    ps,
                    lhsT=a1t[:, k, ts(t, P)],
                    rhs=wa2_sb[:, k, :],
                    start=(k == 0),
                    stop=False,
                )
            nc.tensor.matmul(ps, lhsT=ones_row, rhs=ba2_sb, start=False, stop=True)
            negm = sp.tile([P, 1], F32, tag="negm", name="negm")
            nc.vector.reduce_max(negm, ps, axis=mybir.AxisListType.X, negate=True)
            e = sp.tile([P, RANK], F32, tag="esm", name="esm")
            ssum = sp.tile([P, 1], F32, tag="ssum", name="ssum")
            nc.scalar.activation(e, ps, AF.Exp, bias=negm, scale=1.0, accum_out=ssum)
            rs = sp.tile([P, 1], F32, tag="rs", name="rs")
            nc.vector.reciprocal(rs, ssum)
            beta = gtp.tile([P, RANK], F32, tag=f"beta{t}", name=f"beta{t}")
            nc.vector.tensor_scalar_mul(beta, e, rs)
            betas[t] = beta

        def emit_rank(t):
            beta = betas[t]
            acc = zp.tile([P, 512], F32, tag="acc512", name="acc512")
            for c in range(NCH):
                pz = []
                for m in range(3):
                    ps = pp.tile([P, 512], F32, tag="ps", name="ps_rk")
                    for k in range(KF):
                        nc.tensor.matmul(
                            ps,
                            lhsT=gt[t][m][k],
                            rhs=uvs_sb[:, m * KF + k, ts(c, 512)],
                            start=(k == 0),
                            stop=(k == KF - 1),
                        )
                    pz.append(ps)
                # fold beta into the zg eviction (per-rank scale, on ACT)
                ugb = cp.tile([P, 512], BF16, tag="ugb", name="ugb")
                for rr in range(2):
                    r = 2 * c + rr
                    nc.scalar.activation(
                        ugb[:, ts(rr, D_F)],
                        pz[0][:, ts(rr, D_F)],
                        AF.Copy,
                        scale=beta[:, r : r + 1],
                    )
                u2 = cp.tile([P, 512], BF16, tag="u2", name="u2")
                nc.scalar.activation(u2, pz[1], AF.Copy)
                tm = cp.tile([P, 512], BF16, tag="tm", name="tm")
                nc.gpsimd.tensor_tensor(tm, ugb, u2, op=OP.mult)
                t2 = cp.tile([P, 512], BF16, tag="t2", name="t2")
                nc.vector.tensor_tensor(t2, tm, pz[2], op=OP.mult)
                if c == 0:
                    nc.vector.tensor_copy(acc, t2)
                else:
                    nc.vector.tensor_add(acc, t2, acc)
            zfin = zp.tile([P, D_F], F32, tag="zfin", name="zfin")
            nc.vector.tensor_add(zfin, acc[:, 0:D_F], acc[:, D_F : 2 * D_F])
            nc.sync.dma_start(out=z_out[ts(t, P), :], in_=zfin)

        for c in range(bc // 512):
            emit_attn_l1(c)
            for t in range(4 * c, min(4 * c + 4, nbt)):
                emit_a2_softmax(t)
                emit_gtt(t)
                emit_rank(t)
        cpp.__exit__(None, None, None)

    nc.compile()
    return nc


_BF = ml_dtypes.bfloat16


def _pack_weights(inputs):
    """Host-side offline packing: transpose + cast weights once."""
    f = np.asarray
    wc_t = np.concatenate(
        [f(inputs["Wg"]).T, f(inputs["W2"]).T, f(inputs["W3"]).T], axis=0
    ).astype(_BF)  # [2304, 256]
    uvs_t = np.concatenate(
        [f(inputs["U"]).T, f(inputs["V"]).T, f(inputs["S"]).T], axis=0
    ).astype(_BF)  # [768, 4096]
    wa1_t = np.ascontiguousarray(f(inputs["Wa1"]).T).astype(_BF)  # [2304, 512]
    wa2_t = np.ascontiguousarray(f(inputs["Wa2"]).T).astype(_BF)  # [512, 16]
    consts_b = np.zeros((4, D_F), dtype=_BF)
    consts_b[0] = f(inputs["bg"]).astype(_BF)
    consts_b[1] = f(inputs["b2"]).astype(_BF)
    consts_b[2] = f(inputs["b3"]).astype(_BF)
    consts_b[3, :RANK] = f(inputs["ba2"]).astype(_BF)
    consts_f = np.concatenate(
        [
            f(inputs["ba1"]).reshape(NH, P).T,
            np.concatenate(
                [
                    f(inputs["ln_g_w"]).reshape(KF, P),
                    f(inputs["ln_2_w"]).reshape(KF, P),
                    f(inputs["ln_3_w"]).reshape(KF, P),
                ],
                axis=0,
            ).T,
            np.concatenate(
                [
                    f(inputs["ln_g_b"]).reshape(KF, P),
                    f(inputs["ln_2_b"]).reshape(KF, P),
                    f(inputs["ln_3_b"]).reshape(KF, P),
                ],
                axis=0,
            ).T,
        ],
        axis=1,
    ).astype(np.float32)  # [128, 16]
    return {
        "wc_t": wc_t,
        "uvs_t": uvs_t,
        "wa1_t": wa1_t,
        "wa2_t": wa2_t,
        "consts_f": consts_f,
        "consts_b": consts_b,
    }


_NC_CACHE = {}


def _get_nc():
    if "nc" not in _NC_CACHE:
        _NC_CACHE["nc"] = build_kernel()
    return _NC_CACHE["nc"]


def kernel(run_opts=None, **inputs):
    nc = _get_nc()
    wmap = _pack_weights(inputs)
    h_g = np.ascontiguousarray(np.asarray(inputs["h_g"], dtype=np.float32))
    h_2d = np.ascontiguousarray(np.asarray(inputs["h_2d"], dtype=np.float32))
    h_3d = np.ascontiguousarray(np.asarray(inputs["h_3d"], dtype=np.float32))

    in_maps = []
    for i in range(N_CORES):
        sl = slice(i * BC, (i + 1) * BC)
        m = dict(wmap)
        m["h_g"] = h_g[sl]
        m["h_2d"] = h_2d[sl]
        m["h_3d"] = h_3d[sl]
        in_maps.append(m)

    res = run_bass_kernel_spmd(
        nc, in_maps, core_ids=list(range(N_CORES)), **(run_opts or {})
    )
    out = np.concatenate([r["z"] for r in res.results], axis=0)
    if run_opts:
        kernel.last_results = res
    return out
